# revision 2
# baseline (speedup 1.0000x reference)
"""Trainium2 Bass kernel for nn_FT_init: pixel_unshuffle -> FFT2 -> all-pairs
magnitude/phase recombination -> IFFT2 -> pixel_shuffle.

Strategy: pure data parallel, one sample per NeuronCore (8 cores).

Design (117us baseline -> ~90us):
- Forward FFT2 (two matmul stages, data as stationary), mag/unit-phase pipe
  in fp16 (zs holds z/16; scale folded into the eviction and the stage-2
  const) so all DVE elementwise ops run in 2x mode; sqrt has a +1e-4 bias
  to bound u at near-zero bins; reciprocal_approx_fast (f32, 18 bits).
- Inverse per (i,half): one merged 4096-wide fp16 multiply builds
  [cA|cB] = [uPA|uPB] * mag_i (uPA/uPB block2 signs swapped, M2 rows 65:128
  flipped to compensate); stage 1 = data-stationary matmuls (Hermitian-
  folded 128-col packing); stage 2 = const-stationary M2h with 512-wide
  moving, output transposed [x, j, y] so every PSUM eviction is contiguous;
  the pixel-shuffle interleave happens on the host (numpy view permute).
- Engine budget: DVE = c-mults + 14 output evictions; Act = stage-1
  evictions + remaining output evictions + forward sqrt/magP; Pool stays
  idle (its SBUF traffic halves DVE throughput); one output DMA per i.
- PE warmup matmuls + a dummy sqrt pin the pstate ramp and both act-table
  loads into the input-DMA shadow.
"""
import sys

sys.path.insert(0, "/opt/trn_rl_repo")

import numpy as np
import concourse.bass as bass  # noqa: E402
import concourse.mybir as mybir  # noqa: E402
import concourse.tile as tile  # noqa: E402
import concourse.bacc as bacc  # noqa: E402
from concourse.bass_utils import run_bass_kernel_spmd  # noqa: E402

F32 = mybir.dt.float32
FP16 = mybir.dt.float16
N = 128  # DFT size
R = 4  # msfa / pixel shuffle factor
C = 16  # channels = R*R
MULT = mybir.AluOpType.mult
ADD = mybir.AluOpType.add
SQRT = mybir.ActivationFunctionType.Sqrt
ZSC = 1.0 / 16.0  # zs scale: zs holds z/16 (fp16 range headroom)


def _dft_consts():
    k = np.arange(N)
    ang = 2.0 * np.pi / N * np.outer(k, k)
    Wr = np.cos(ang).astype(np.float32)
    Wi = (-np.sin(ang)).astype(np.float32)
    Er = (np.cos(ang) / N).astype(np.float32)
    Ei = (np.sin(ang) / N).astype(np.float32)
    # stage-2 folded const: out[y,x] = sum_k s[k,y] * M2[k,x] with
    # s = [S1r(n=0..64) | S1i(n=1..63)] (block2 sign-swapped, see uPA/uPB)
    w = np.ones(65, np.float32)
    w[1:64] = 2.0
    M2 = np.zeros((N, N), np.float32)
    M2[0:65] = w[:, None] * Er[0:65]
    M2[65:128] = 2.0 * Ei[1:64]
    M2 *= 1.0 / ZSC  # compensate the zs scale
    cinh = np.hstack([
        Wr, Wi,                      # WWh      [0:256]
        Wr[:, 0:65], Wi[:, 0:65],    # WB1      [256:386]
        -Wi[:, 0:65], Wr[:, 0:65],   # WB2      [386:516]
        Er,                          # Erh      [516:644]
        -Ei,                         # Ginh     [644:772]
        M2,                          # M2h      [772:900]
    ]).astype(np.float16)
    return cinh


def _build():
    nc = bacc.Bacc("TRN2", target_bir_lowering=False, debug=False, num_devices=8)
    xin = nc.dram_tensor("xin", [128, 2048], FP16, kind="ExternalInput")
    cinh = nc.dram_tensor("cinh", [128, 900], FP16, kind="ExternalInput")
    # outd[i, x, half*1024 + jj*128 + y]  (j = half*8 + jj)
    outd = nc.dram_tensor("outd", [C, 128, 2048], FP16, kind="ExternalOutput")

    with tile.TileContext(nc) as tc:
        with (
            tc.tile_pool(name="persist", bufs=1) as pp,
            tc.tile_pool(name="sx", bufs=3) as sxp,
            tc.tile_pool(name="cpool", bufs=2) as cpool,
            tc.tile_pool(name="s4pool", bufs=3) as s4p,
            tc.tile_pool(name="ohpool", bufs=3) as ohp,
            tc.tile_pool(name="ps1", bufs=2, space="PSUM") as ps1,
            tc.tile_pool(name="ps2", bufs=2, space="PSUM") as ps2,
        ):
            xrows = pp.tile([128, 2048], FP16)
            consts = pp.tile([128, 900], FP16)
            # input chunks on sync queue; consts concurrently on Act queue
            nc.sync.dma_start(xrows[:, 0:1024], xin[:, 0:1024])
            nc.scalar.dma_start(consts[:, 0:256], cinh[:, 0:256])
            nc.sync.dma_start(xrows[:, 1024:2048], xin[:, 1024:2048])
            nc.scalar.dma_start(consts[:, 256:900], cinh[:, 256:900])
            WWh = consts[:, 0:256]
            WB1 = consts[:, 256:386]
            WB2 = consts[:, 386:516]
            Erh = consts[:, 516:644]
            Ginh = consts[:, 644:772]
            M2h = consts[:, 772:900]

            def apv(t, off, dims):
                return bass.AP(t[:].tensor, t[:].offset + off,
                               [t[:].ap[0]] + dims)

            biast = pp.tile([128, 1], F32)
            nc.gpsimd.memset(biast[:], 1e-4)

            warm = pp.tile([128, 128], FP16)
            nc.gpsimd.memset(warm[:], 0.0)
            wsc = pp.tile([128, 1], F32)
            # force the sqrt act table to load first (Copy lives in every
            # table, so no second ACT_TABLE_LOAD mid-forward)
            nc.scalar.activation(wsc[:], biast[:], SQRT)
            psW = ps1.tile([128, 512], F32, tag="s1")
            for _w in range(28):
                nc.tensor.matmul(psW[:, 0:128], warm[:], warm[:],
                                 start=True, stop=True)

            # zs: per channel c, 130 cols: [zr(n=0..64) | zi(n=0..64)] / 16
            zs = pp.tile([128, 2080], FP16)
            t1 = pp.tile([128, 1040], FP16)
            t2 = pp.tile([128, 1040], FP16)
            sq = pp.tile([128, 1040], FP16)
            mag_f = pp.tile([128, 1040], F32)
            rmag = pp.tile([128, 1040], F32)
            rmag16 = pp.tile([128, 1040], FP16)
            rmagn16 = pp.tile([128, 1040], FP16)
            uPAB = pp.tile([128, 4096], FP16)
            magP = pp.tile([128, 2048], FP16)

            # ---- forward FFT2 (fp16), 2 channels per group ----
            sx_t = [None] * 8

            def fwd_A(g):
                psA = ps1.tile([128, 512], F32, tag="s1")
                for cc in range(2):
                    c = g * 2 + cc
                    p, q = divmod(c, R)
                    xs = bass.AP(
                        xrows[:].tensor,
                        xrows[:].offset + p * 512 + q,
                        [xrows[:].ap[0], [4, 128]],
                    )
                    nc.tensor.matmul(psA[:, cc * 256:(cc + 1) * 256], xs, WWh,
                                     start=True, stop=True)
                sx = sxp.tile([128, 512], FP16)
                sx_t[g] = sx
                if g % 2 == 0:
                    nc.vector.tensor_copy(sx[:], psA[:])
                else:
                    nc.scalar.copy(sx[:], psA[:])

            def fwd_B(g):
                sx = sx_t[g]
                psB = ps2.tile([128, 512], F32, tag="s2")
                for cc in range(2):
                    o = cc * 256
                    zo = cc * 130
                    nc.tensor.matmul(psB[:, zo:zo + 130], sx[:, o:o + 128],
                                     WB1, start=True, stop=False)
                    nc.tensor.matmul(psB[:, zo:zo + 130], sx[:, o + 128:o + 256],
                                     WB2, start=False, stop=True)
                # evict z/16 (fp16): [Zr(0:65)|Zi(0:65)] x 2 chans
                nc.scalar.mul(zs[:, g * 260:(g + 1) * 260],
                              psB[:, 0:260], ZSC)

            # ---- magnitude / unit phase for one half (8 channels) ----
            def pipe_half(h):
                zo = h * 1040
                co = h * 520
                uo = h * 1024
                zr_v = apv(zs, zo, [[130, 8], [1, 65]])
                zi_v = apv(zs, zo + 65, [[130, 8], [1, 65]])
                t1v = apv(t1, co, [[65, 8], [1, 65]])
                t2v = apv(t2, co, [[65, 8], [1, 65]])
                nc.vector.tensor_tensor(t1v, zr_v, zr_v, MULT)
                nc.vector.tensor_tensor(t2v, zi_v, zi_v, MULT)
                nc.vector.tensor_tensor(sq[:, co:co + 520], t1[:, co:co + 520],
                                        t2[:, co:co + 520], ADD)
                # mag_f = sqrt(sq + 1e-4): f32 out for the reciprocal; the
                # bias bounds u at near-zero spectrum bins.
                nc.scalar.activation(mag_f[:, co:co + 520], sq[:, co:co + 520],
                                     SQRT, bias=biast[:])
                nc.vector.reciprocal_approx_fast(
                    rmag[:, co:co + 520], mag_f[:, co:co + 520])
                nc.vector.tensor_copy(rmag16[:, co:co + 520],
                                      rmag[:, co:co + 520])
                nc.vector.tensor_scalar_mul(rmagn16[:, co:co + 520],
                                            rmag16[:, co:co + 520], -1.0)
                rm_v = apv(rmag16, co, [[65, 8], [1, 65]])
                rm1_v = apv(rmag16, co + 1, [[65, 8], [1, 63]])
                rmn1_v = apv(rmagn16, co + 1, [[65, 8], [1, 63]])
                # packed unit-phase tiles (fp16), block2 signs swapped
                # (M2 rows 65:128 flipped to compensate):
                # uPA[j-block] = [ur(0:65) | -ui(1:64)]
                # uPB[j-block] = [ui(0:65) |  ur(1:64)]
                nc.vector.tensor_tensor(
                    apv(uPAB, uo, [[128, 8], [1, 65]]), zr_v, rm_v, MULT)
                nc.vector.tensor_tensor(
                    apv(uPAB, uo + 65, [[128, 8], [1, 63]]),
                    apv(zs, zo + 66, [[130, 8], [1, 63]]), rmn1_v, MULT)
                nc.vector.tensor_tensor(
                    apv(uPAB, 2048 + uo, [[128, 8], [1, 65]]), zi_v, rm_v, MULT)
                nc.vector.tensor_tensor(
                    apv(uPAB, 2048 + uo + 65, [[128, 8], [1, 63]]),
                    apv(zs, zo + 1, [[130, 8], [1, 63]]), rm1_v, MULT)
                # packed magnitudes (scaled): [mag(0:65) | mag(1:64)]
                nc.scalar.copy(
                    apv(magP, uo, [[128, 8], [1, 65]]),
                    apv(mag_f, co, [[65, 8], [1, 65]]))
                nc.scalar.copy(
                    apv(magP, uo + 65, [[128, 8], [1, 63]]),
                    apv(mag_f, co + 1, [[65, 8], [1, 63]]))

            for g in range(8):
                fwd_A(g)
                if g >= 1:
                    fwd_B(g - 1)
            fwd_B(7)
            pipe_half(0)
            pipe_half(1)

            # ---- inverse: per magnitude channel i, software-pipelined ----
            s4_t = [None] * 32

            def inv_mults(i):
                mb = bass.AP(magP[:].tensor, magP[:].offset + i * 128,
                             [magP[:].ap[0], [0, 2 * C], [1, 128]])
                cAB = cpool.tile([128, 4096], FP16, tag="cAB")
                cv = cAB[:].rearrange("z (c n) -> z c n", n=128)
                uv = uPAB[:].rearrange("z (c n) -> z c n", n=128)
                nc.vector.tensor_tensor(cv, uv, mb, MULT)
                return cAB

            def inv_stage1(k, cAB):
                half = k % 2
                co = half * 1024
                ps1t = ps1.tile([128, 1024], F32, tag="s1")
                for jj in range(8):
                    o = co + jj * 128
                    nc.tensor.matmul(ps1t[:, jj * 128:jj * 128 + 128],
                                     cAB[:, o:o + 128], Erh,
                                     start=True, stop=False)
                    nc.tensor.matmul(ps1t[:, jj * 128:jj * 128 + 128],
                                     cAB[:, 2048 + o:2048 + o + 128], Ginh,
                                     start=False, stop=True)
                s4 = s4p.tile([128, 1024], FP16)
                s4_t[k] = s4
                nc.scalar.copy(s4[:], ps1t[:])

            oh_t = [None]

            def inv_stage2(k):
                i, half = divmod(k, 2)
                s4 = s4_t[k]
                # stage 2: stationary = M2h (const), moving = s4 (wide)
                # out[x, jj*128 + y] transposed; interleave done on host
                ps2t = ps2.tile([128, 1024], F32, tag="s2")
                nc.tensor.matmul(ps2t[:, 0:512], M2h, s4[:, 0:512],
                                 start=True, stop=True)
                nc.tensor.matmul(ps2t[:, 512:1024], M2h, s4[:, 512:1024],
                                 start=True, stop=True)
                if half == 0:
                    oh_t[0] = ohp.tile([128, 2048], FP16, name="oh")
                oh = oh_t[0]
                dst = oh[:, half * 1024:(half + 1) * 1024]
                if k % 2 == 1 and k not in (15, 31):
                    nc.vector.tensor_copy(dst, ps2t[:])
                else:
                    nc.scalar.copy(dst, ps2t[:])
                if half == 1:
                    nc.sync.dma_start(outd[i, :, :], oh[:])

            cAB = None
            for k in range(32):
                if k % 2 == 0:
                    cAB = inv_mults(k // 2)
                inv_stage1(k, cAB)
                if k >= 1:
                    inv_stage2(k - 1)
            inv_stage2(31)

    nc.compile()
    return nc


_NC = None


def _get_nc():
    global _NC
    if _NC is None:
        _NC = _build()
    return _NC


def _unshuffle_host(o):
    # o: [C, 128, 2048] = [i, x, (half, jjh, ss, y)] -> [C, 512, 512]
    a = o.reshape(C, 128, 2, 2, 4, 128)          # i, x, half, jjh, ss, y
    a = a.transpose(0, 5, 2, 3, 1, 4)            # i, y, half, jjh, x, ss
    return np.ascontiguousarray(a).reshape(C, 512, 512)


def kernel(x: np.ndarray) -> np.ndarray:
    x = np.asarray(x, dtype=np.float32)
    assert x.shape == (8, 1, 512, 512), x.shape
    nc = _get_nc()
    cinh = _dft_consts()
    in_maps = [
        {"xin": np.ascontiguousarray(x[b, 0].reshape(128, 2048)).astype(np.float16),
         "cinh": cinh}
        for b in range(8)
    ]
    res = run_bass_kernel_spmd(nc, in_maps, core_ids=list(range(8)))
    out = np.stack([_unshuffle_host(r["outd"].astype(np.float32))
                    for r in res.results])
    return out


if __name__ == "__main__":
    rng = np.random.RandomState(0)
    x = rng.randn(8, 1, 512, 512).astype(np.float32)
    y = kernel(x)
    print(y.shape, y.dtype)
